# revision 18
# baseline (speedup 1.0000x reference)
"""DenseKANLayer Trainium2 kernel.

Math: for each edge e=(o,i), the reference computes a cubic B-spline
s_e(x) = sum_g c_basis[e,g] * B_{e,g}(x) on the 15-point knot row of e,
then y[b,o] = sum_i c_spl[o,i]*s_(o,i)(x[b,i]) + c_res[o,i]*silu(x[b,i]) + bias[o].

A cubic B-spline combination is exactly a sum of truncated powers:
s_e(x) = sum_m beta[e,m] * relu(x - t_m)^3 (beta = jump of s'''/6 per knot,
computed host-side in float64).  Knots with t_m <= min_b x[b,i] never
truncate, so their terms are plain cubics: folded host-side into 4 "poly"
channels {1, x, x^2, x^3} (the constant one merges with bias).  Knots with
t_m >= max x contribute nothing.  What remains is a small set of true relu
features.  Folding c_spl gives one PSUM-accumulated matmul stack:

    y.T = sum_c Wc @ Phi_c,   channels c = [x, x^2, x^3, ones(=bias),
                                            relu-cubes..., silu]

Precision: the truncated-power basis is ill-conditioned (terms ~30x the
result cancel in the fp32 PSUM), so W and Phi need ~fp32 for channels with
large feature amplitude.  Channels whose max |x - t|^3 is small (knots near
the top of the data range) and the silu channel are safe in fp16 (measured
rel err ~3e-3 vs the 2e-2 budget; fp16 keeps the 2-byte DVE fast modes and
1-cycle/row PE rate).

Per-core program (batch sharded 8 ways, S=64 columns/core):
- SP:   DMA1 xp = [x.T shard | -t scalars | pad] (128x128 f32, the
        latency-critical load), DMA W16 [silu + small relu chunks] (f16),
        then the output DMA (see race note below).
- Pool: DMA W32a [x,x^2,x^3,bias] via SWDGE (own descriptor unit, so its
        gen overlaps the HWDGE gens), memset ONES, x^2/x^3 products, and
        the final PSUM->SBUF copy.
- ACT:  DMA W32b [big relu chunks], silu table load, silu (f16 out),
        x->f16 cast for the f16 relu features.
- DVE:  fp32 relu+square+cube chain for the big chunks, f16 chain (4x/2x
        modes) for the small ones.
- PE:   warmup matmul (ramps the clock gate), then the chunk matmuls
        accumulating in PSUM, bias folded in via the ONES channel.

Output-DMA race (deliberate, bounded): the out DMA is issued when the f16
squares finish (s_gate), ~600ns of compute before Y is final, while the
descriptor-gen + DGE pipeline takes ~1.3us before the engines read Y.
Same pattern the previous baseline shipped (bias-add vs descriptor-gen),
with a wider margin measured in the cost model (~600ns).
"""

import numpy as np

N_IN, N_OUT, SPLINE_K, G = 128, 128, 3, 8
BATCH = 512
EDGES = N_IN * N_OUT
N_KNOTS = G + 2 * SPLINE_K + 1          # 15
N_CORES = 8
BSHARD = BATCH // N_CORES               # 64
# relu channels whose max |x-t|^3 is below this go to fp16
F16_AMP_THRESH = 1.1

_COMPILED = {}


def _beta_from_bspline(knots, c_basis):
    """Truncated-power coefficients beta (EDGES, N_KNOTS) such that
    sum_g c[e,g] B_{e,g}(x) == sum_m beta[e,m] relu(x - knots[e,m])^3
    exactly (computed in float64)."""
    E = knots.shape[0]
    t = knots.astype(np.float64)
    c = c_basis.astype(np.float64)

    def deriv(c, k):
        m = c.shape[1]
        cpad = np.concatenate(
            [np.zeros((E, 1)), c, np.zeros((E, 1))], axis=1)
        g = np.arange(m + 1)
        denom = t[:, g + k] - t[:, g]
        with np.errstate(divide="ignore", invalid="ignore"):
            d = k * (cpad[:, 1:] - cpad[:, :-1]) / denom
        return np.where(denom == 0, 0.0, d)

    c3 = deriv(deriv(deriv(c, 3), 2), 1)          # s''' per interval (E, 14)
    c3pad = np.concatenate([np.zeros((E, 1)), c3, np.zeros((E, 1))], axis=1)
    return (c3pad[:, 1:] - c3pad[:, :-1]) / 6.0   # (E, 15)


def _reference_numpy(x, knots, c_basis, c_spl, c_res, bias):
    """Exact (slow) fallback for inputs the factorized kernel can't
    handle (knot rows differing across the n_out axis)."""
    batch = x.shape[0]
    x_ext = np.broadcast_to(x[:, None, :], (batch, N_OUT, N_IN)).reshape(batch, EDGES).T
    grid = knots[:, :, None]
    x_in = x_ext[:, None, :]
    b = ((x_in >= grid[:, :-1]) & (x_in < grid[:, 1:])).astype(np.float32)
    for order in range(1, SPLINE_K + 1):
        n0 = grid[:, order:-1] - grid[:, :-(order + 1)]
        n1 = grid[:, order + 1:] - grid[:, 1:-order]
        with np.errstate(divide="ignore", invalid="ignore"):
            left = np.where(n0 == 0, 0.0, (x_in - grid[:, :-(order + 1)]) / n0)
            right = np.where(n1 == 0, 0.0, (grid[:, order + 1:] - x_in) / n1)
        b = left * b[:, :-1] + right * b[:, 1:]
    spl = np.einsum("eg,egb->eb", c_basis, b).T
    y = c_spl.reshape(1, EDGES) * spl
    sig = 1.0 / (1.0 + np.exp(-x_ext.T))
    y = y + c_res.reshape(1, EDGES) * (x_ext.T * sig)
    return (y.reshape(batch, N_OUT, N_IN).sum(axis=2) + bias).astype(np.float32)


def _build_program(nf32, nf16):
    """Raw-bacc per-core program.

    Channels: 4 fp32 poly [x, x^2, x^3, ones] + nf32 fp32 relu-cubes
    + fp16 [silu] + nf16 fp16 relu-cubes.  nf32 >= 1, nf16 >= 0.
    """
    import concourse.bass as bass
    import concourse.mybir as mybir
    from concourse import bacc
    from concourse.hw_specs import get_activation_tables

    S = BSHARD
    NCH32 = 3 + nf32                    # fp32 chunks in W32
    NCH16 = 1 + nf16                    # fp16 chunks in W16
    dt32 = mybir.dt.float32
    dt16 = mybir.dt.float16
    ACTF = mybir.ActivationFunctionType
    ALU = mybir.AluOpType

    class SlimBlock(bass.BassBlock):
        """Skip the exit drain + all-engine barrier; completion is carried
        by the explicit semaphore chain ending in s_y."""
        def __exit__(self, exc_type, exc_val, exc_tb):
            if exc_type is None:
                for engine, last_body in self.last_body.items():
                    with self.bass.body(last_body, parent=self.bass.cur_bb,
                                        allow_existing_parent=True):
                        engine.br(self.end_bb)
                self.bass.switch_bb(self.end_bb)

    class FastBacc(bacc.Bacc):
        """Skip the constructor's const-AP entry barrier: nothing reads the
        const tiles before a much-later semaphore wait."""
        _skip_entry_barrier = True

        def all_engine_barrier(self, **kw):
            if getattr(self, "_skip_entry_barrier", False):
                return
            return super().all_engine_barrier(**kw)

    # Bass.__init__ seeds four const tiles with Pool memsets.  None of
    # them needs real initialization here: the silu bias reads a zero
    # column of xp instead of the const-zero tile, and the warmup matmul
    # result is discarded.  The memsets would delay Pool's SWDGE
    # descriptor-gen by ~300ns, which delays the first W transfer.  The
    # patch goes on BassGpSimd (not BassSharedVectorInterface) so it wins
    # the MRO over the Rust base class's memset.
    _had_own = "memset" in vars(bass.BassGpSimd)
    _orig_memset = bass.BassGpSimd.memset

    def _filtered_memset(self, ap, constant):
        return None

    bass.BassGpSimd.memset = _filtered_memset
    try:
        nc = FastBacc("TRN2", target_bir_lowering=False, debug=False)
    finally:
        if _had_own:
            bass.BassGpSimd.memset = _orig_memset
        else:
            del bass.BassGpSimd.memset
    nc._skip_entry_barrier = False

    xp = nc.dram_tensor("xp", [128, 128], dt32, kind="ExternalInput")
    w32 = nc.dram_tensor("w32", [N_IN, NCH32 * N_OUT], dt32, kind="ExternalInput")
    w16 = nc.dram_tensor("w16", [N_IN, NCH16 * N_OUT], dt16, kind="ExternalInput")
    y = nc.dram_tensor("y", [N_OUT, S], dt32, kind="ExternalOutput")

    from contextlib import ExitStack
    with ExitStack() as stack:
        ent = stack.enter_context
        XP = ent(nc.sbuf_tensor([128, 128], dt32))
        W32 = ent(nc.sbuf_tensor([N_IN, NCH32 * N_OUT], dt32))
        W16 = ent(nc.sbuf_tensor([N_IN, NCH16 * N_OUT], dt16))
        PHI32 = ent(nc.sbuf_tensor([128, 2 * S], dt32))    # [x^2 | x^3]
        R32 = ent(nc.sbuf_tensor([128, max(nf32, 1) * S], dt32))
        SQ32 = ent(nc.sbuf_tensor([128, max(nf32, 1) * S], dt32))
        XH = ent(nc.sbuf_tensor([128, S], dt16))
        PHI16 = ent(nc.sbuf_tensor([128, NCH16 * S], dt16))  # [silu | cubes]
        SQ16 = ent(nc.sbuf_tensor([128, max(nf16, 1) * S], dt16))
        Y = ent(nc.sbuf_tensor([N_OUT, S], dt32))
        acc = ent(nc.psum_tensor([N_OUT, S], dt32))
        dump = ent(nc.psum_tensor([1, 512], dt32))
        s_pack = ent(nc.semaphore("s_pack"))
        s_wa = ent(nc.semaphore("s_wa"))
        s_wb = ent(nc.semaphore("s_wb"))
        s_wc = ent(nc.semaphore("s_wc"))
        s_pool = ent(nc.semaphore("s_pool"))
        s_silu = ent(nc.semaphore("s_silu"))
        s_xh = ent(nc.semaphore("s_xh"))
        s_cub1 = ent(nc.semaphore("s_cub1"))
        s_cub2 = ent(nc.semaphore("s_cub2"))
        s_mm = ent(nc.semaphore("s_mm"))
        s_cp = ent(nc.semaphore("s_cp"))
        s_y = ent(nc.semaphore("s_y"))
        block = ent(SlimBlock(nc, "main"))
        X = XP[:, :S]
        NTA = XP[:, S:S + nf32]                  # -t for fp32 relu chunks
        NTB = XP[:, S + nf32:S + nf32 + nf16]    # -t for fp16 relu chunks
        BIAS = XP[:, S + nf32 + nf16:S + nf32 + nf16 + 1]  # folded bias per o
        W32A_COLS = 3 * N_OUT                    # [x, x^2, x^3]

        @block.sync
        def _(sp):
            sp.dma_start(out=XP[:], in_=xp.ap()).then_inc(s_pack, 16)
            sp.dma_start(out=W16[:], in_=w16.ap()).then_inc(s_wc, 16)
            # Strictly after the bias-add copy lands in Y.  (A same-tick
            # race gating this on s_mm looks attractive -- the ~1.3us
            # descriptor-gen pipeline vs the ~450ns copy -- but on a cold
            # device the first-ever execution reads stale SBUF: the runtime
            # does not order a DMA read after a same-tick engine write.)
            sp.wait_ge(s_cp, 1)
            sp.dma_start(out=y.ap(), in_=Y[:]).then_inc(s_y, 16)
            sp.wait_ge(s_y, 16)

        @block.gpsimd
        def _(pool):
            pool.dma_start(out=W32[:, :W32A_COLS],
                           in_=w32.ap()[:, :W32A_COLS]).then_inc(s_wa, 16)
            pool.wait_ge(s_pack, 16)
            nc.gpsimd.tensor_mul(PHI32[:, :S], X, X)
            nc.gpsimd.tensor_mul(PHI32[:, S:2 * S], PHI32[:, :S], X) \
                .then_inc(s_pool, 1)

        @block.scalar
        def _(act):
            act.dma_start(out=W32[:, W32A_COLS:],
                          in_=w32.ap()[:, W32A_COLS:]).then_inc(s_wb, 16)
            tabs = get_activation_tables(nc.m.arch)
            set_id = list(tabs).index("silu_and_others")
            ld = mybir.InstLoadActFuncSet(
                name=nc.get_next_instruction_name(), ins=[], outs=[],
                act_func_set_id=set_id)
            ld.engine = mybir.EngineType.Activation
            nc.scalar.add_instruction(ld)
            act.wait_ge(s_pack, 16)
            nc.scalar.activation(PHI16[:, :S], X, ACTF.Silu,
                                 bias=XP[:, 127:128]).then_inc(s_silu, 1)
            nc.scalar.activation(XH[:], X, ACTF.Copy).then_inc(s_xh, 1)
            act.wait_ge(s_mm, 1)
            nc.scalar.activation(Y[:], acc[:], ACTF.Identity,
                                 bias=BIAS).then_inc(s_cp, 1)

        @block.vector
        def _(dve):
            dve.wait_ge(s_pack, 16)
            for j in range(nf32):
                nc.vector.tensor_scalar(
                    R32[:, j * S:(j + 1) * S], X, NTA[:, j:j + 1], 0.0,
                    op0=ALU.add, op1=ALU.max)
            g32 = nf32 * S
            nc.vector.tensor_mul(SQ32[:, :g32], R32[:, :g32], R32[:, :g32])
            nc.vector.tensor_mul(R32[:, :g32], SQ32[:, :g32], R32[:, :g32]) \
                .then_inc(s_cub1, 1)
            if nf16:
                dve.wait_ge(s_xh, 1)
                for j in range(nf16):
                    nc.vector.tensor_scalar(
                        PHI16[:, (1 + j) * S:(2 + j) * S], XH[:],
                        NTB[:, j:j + 1], 0.0, op0=ALU.add, op1=ALU.max)
                g16 = nf16 * S
                RH = PHI16[:, S:S + g16]
                nc.vector.tensor_mul(SQ16[:, :g16], RH, RH)
                nc.vector.tensor_mul(RH, SQ16[:, :g16], RH).then_inc(s_cub2, 1)


        @block.tensor
        def _(pe):
            # Discarded wide matmul: keeps the PE HAM clock-gate ramping
            # during the DMA wait so the real matmuls run at 2.4 GHz
            # instead of the cold 1.2 GHz (reads whatever is in W32; the
            # result is never consumed).
            zc = nc.const_aps.aps[(mybir.dt.float32, 0.0)]
            nc.tensor.matmul(dump[:], zc, W32[:, :512],
                             start=True, stop=True, skip_group_check=True)
            # Second warmup: carries the busy streak through the W-DMA wait
            # so the real matmuls run at the full 2.4GHz p-state (a ~1.8us
            # idle gap demotes them to the mid p-state, 2x slower).
            nc.tensor.matmul(dump[:, :64], zc, W32[:, :64],
                             start=True, stop=True, skip_group_check=True)
            pe.wait_ge(s_wa, 16)
            pe.wait_ge(s_pack, 16)
            pe.wait_ge(s_pool, 1)
            nc.tensor.matmul(acc[:], W32[:, :N_OUT], X, start=True, stop=False)
            nc.tensor.matmul(acc[:], W32[:, N_OUT:2 * N_OUT], PHI32[:, :S],
                             start=False, stop=False)
            nc.tensor.matmul(acc[:], W32[:, 2 * N_OUT:3 * N_OUT],
                             PHI32[:, S:2 * S], start=False, stop=False)
            pe.wait_ge(s_wb, 16)
            pe.wait_ge(s_cub1, 1)
            for j in range(nf32):
                nc.tensor.matmul(acc[:], W32[:, (3 + j) * N_OUT:(4 + j) * N_OUT],
                                 R32[:, j * S:(j + 1) * S],
                                 start=False, stop=False)
            pe.wait_ge(s_wc, 16)
            pe.wait_ge(s_silu, 1)
            mm = nc.tensor.matmul(acc[:], W16[:, :N_OUT], PHI16[:, :S],
                                  start=False, stop=(nf16 == 0))
            if nf16:
                pe.wait_ge(s_cub2, 1)
                for j in range(nf16):
                    mm = nc.tensor.matmul(
                        acc[:], W16[:, (1 + j) * N_OUT:(2 + j) * N_OUT],
                        PHI16[:, (1 + j) * S:(2 + j) * S],
                        start=False, stop=(j == nf16 - 1))
            mm.then_inc(s_mm, 1)

    nc.compile()
    return nc


def kernel(x, knots, c_basis, c_spl, c_res, bias):
    x = np.asarray(x, np.float32)
    knots = np.asarray(knots, np.float32)
    c_basis = np.asarray(c_basis, np.float32)
    c_spl = np.asarray(c_spl, np.float32)
    c_res = np.asarray(c_res, np.float32)
    bias = np.asarray(bias, np.float32)

    # Factorization requires the knot row to be shared across the n_out
    # axis for each input column i (true for the reference's broadcast
    # knots).  Otherwise fall back to the exact host implementation.
    kr = knots.reshape(N_OUT, N_IN, N_KNOTS)
    t_col = kr[0]                                     # (N_IN, N_KNOTS)
    if not np.array_equal(kr, np.broadcast_to(t_col[None], kr.shape)):
        return _reference_numpy(x, knots, c_basis, c_spl, c_res, bias)
    # Truncated powers don't vanish past the last knot (where the
    # reference's B-spline support ends), so x beyond it needs the
    # exact path.
    if np.any(x >= t_col[:, -1][None, :]):
        return _reference_numpy(x, knots, c_basis, c_spl, c_res, bias)

    beta = _beta_from_bspline(knots, c_basis)         # (EDGES, 15) f64
    xmin = x.min(axis=0)                              # (N_IN,)
    xmax = x.max(axis=0)

    # Per-edge split: knots with t <= xmin(i) never truncate -> fold into
    # the poly channels; t >= xmax(i) contribute nothing; the rest are
    # relu features, chunked by knot index m.
    ii = np.tile(np.arange(N_IN), N_OUT)
    polyM = knots <= xmin[ii][:, None]                # (EDGES, 15)
    activeM = knots < xmax[ii][:, None]
    reluM = activeM & ~polyM
    relu_cols = np.nonzero(reluM.any(axis=0))[0]

    # Poly coefficients sum_m beta*(x-t)^3 -> a0..a3 per edge (float64).
    a4 = np.zeros((EDGES, 4))
    for m in range(N_KNOTS):
        b = beta[:, m] * polyM[:, m]
        t = knots[:, m].astype(np.float64)
        a4[:, 0] += b * (-t ** 3)
        a4[:, 1] += b * (3 * t ** 2)
        a4[:, 2] += b * (-3 * t)
        a4[:, 3] += b
    cs = c_spl.reshape(EDGES).astype(np.float64)
    Wpoly = cs[:, None] * a4                          # (EDGES, 4)
    bias_fold = bias.astype(np.float64) + \
        Wpoly[:, 0].reshape(N_OUT, N_IN).sum(axis=1)  # a0 folded into bias
    Wrelu = (cs[:, None] * beta)[:, relu_cols] * reluM[:, relu_cols]

    # fp32 vs fp16 relu chunks: fp16 only where the max feature amplitude
    # (xmax - t)^3 stays small, so the ill-conditioned truncated-power
    # cancellation keeps ~fp32 headroom (measured rel err ~3e-3).
    amp = np.maximum(xmax[:, None] - t_col[:, relu_cols], 0.0) ** 3  # (N_IN, R)
    is16 = amp.max(axis=0) <= F16_AMP_THRESH
    cols32 = relu_cols[~is16]
    cols16 = relu_cols[is16]
    nf32, nf16 = len(cols32), len(cols16)
    if nf32 == 0 and len(relu_cols):
        # program requires >= 1 fp32 relu chunk; promote the largest
        cols32 = relu_cols[-1:]
        cols16 = relu_cols[:-1]
        nf32, nf16 = 1, len(cols16)
    elif nf32 == 0:
        # no relu features at all: use a dummy all-zero fp32 chunk
        cols32 = np.array([0])
        nf32 = 1
        Wrelu = np.zeros((EDGES, 1))
        relu_cols = cols32

    def chunkT(w):                                    # (EDGES,) -> (N_IN, N_OUT)
        return np.ascontiguousarray(
            w.reshape(N_OUT, N_IN).T.astype(np.float32))

    WrFull = np.zeros((EDGES, N_KNOTS))
    WrFull[:, relu_cols] = Wrelu
    w32_chunks = [chunkT(Wpoly[:, 1]), chunkT(Wpoly[:, 2]), chunkT(Wpoly[:, 3])]
    for m in cols32:
        w32_chunks.append(chunkT(WrFull[:, m]))
    w32_host = np.ascontiguousarray(
        np.concatenate(w32_chunks, axis=1), dtype=np.float32)
    w16_chunks = [np.ascontiguousarray(c_res.T.astype(np.float16))]
    for m in cols16:
        w16_chunks.append(chunkT(WrFull[:, m]).astype(np.float16))
    w16_host = np.ascontiguousarray(
        np.concatenate(w16_chunks, axis=1), dtype=np.float16)

    # xp pack: [x.T shard | -t(fp32 chunks) | -t(fp16 chunks) | bias | pad]
    nt32 = np.ascontiguousarray(-t_col[:, cols32], dtype=np.float32)
    nt16 = np.ascontiguousarray(-t_col[:, cols16], dtype=np.float32)
    xT = np.ascontiguousarray(x.T, dtype=np.float32)  # (N_IN, BATCH)
    npad = 128 - BSHARD - nf32 - nf16
    pad = np.zeros((N_IN, npad), np.float32)
    pad[:, 0] = bias_fold.astype(np.float32)  # per-partition (= per-o) bias

    key = (nf32, nf16)
    if key not in _COMPILED:
        _COMPILED[key] = _build_program(nf32, nf16)
    nc = _COMPILED[key]

    from concourse.bass_utils import run_bass_kernel_spmd
    core_ids = list(range(N_CORES))
    in_maps = []
    for c in core_ids:
        pack = np.concatenate(
            [xT[:, c * BSHARD:(c + 1) * BSHARD], nt32, nt16, pad], axis=1)
        in_maps.append({"xp": np.ascontiguousarray(pack),
                        "w32": w32_host, "w16": w16_host})
    res = run_bass_kernel_spmd(nc, in_maps, core_ids)
    y_oT = np.concatenate([res.results[c]["y"] for c in core_ids], axis=1)
    return np.ascontiguousarray(y_oT.T, dtype=np.float32)


# revision 19
# speedup vs baseline: 1.0143x; 1.0143x over previous
"""DenseKANLayer Trainium2 kernel.

Math: for each edge e=(o,i), the reference computes a cubic B-spline
s_e(x) = sum_g c_basis[e,g] * B_{e,g}(x) on the 15-point knot row of e,
then y[b,o] = sum_i c_spl[o,i]*s_(o,i)(x[b,i]) + c_res[o,i]*silu(x[b,i]) + bias[o].

A cubic B-spline combination is exactly a sum of truncated powers:
s_e(x) = sum_m beta[e,m] * relu(x - t_m)^3 (beta = jump of s'''/6 per knot,
computed host-side in float64).  Knots with t_m <= min_b x[b,i] never
truncate, so their terms are plain cubics: folded host-side into 4 "poly"
channels {1, x, x^2, x^3} (the constant one merges with bias).  Knots with
t_m >= max x contribute nothing.  What remains is a small set of true relu
features.  Folding c_spl gives one PSUM-accumulated matmul stack:

    y.T = sum_c Wc @ Phi_c,   channels c = [x, x^2, x^3, ones(=bias),
                                            relu-cubes..., silu]

Precision: the truncated-power basis is ill-conditioned (terms ~30x the
result cancel in the fp32 PSUM), so W and Phi need ~fp32 for channels with
large feature amplitude.  Channels whose max |x - t|^3 is small (knots near
the top of the data range) and the silu channel are safe in fp16 (measured
rel err ~3e-3 vs the 2e-2 budget; fp16 keeps the 2-byte DVE fast modes and
1-cycle/row PE rate).

Per-core program (batch sharded 8 ways, S=64 columns/core):
- SP:   DMA1 xp = [x.T shard | -t scalars | pad] (128x128 f32, the
        latency-critical load), DMA W16 [silu + small relu chunks] (f16),
        then the output DMA (see race note below).
- Pool: DMA W32a [x,x^2,x^3,bias] via SWDGE (own descriptor unit, so its
        gen overlaps the HWDGE gens), memset ONES, x^2/x^3 products, and
        the final PSUM->SBUF copy.
- ACT:  DMA W32b [big relu chunks], silu table load, silu (f16 out),
        x->f16 cast for the f16 relu features.
- DVE:  fp32 relu+square+cube chain for the big chunks, f16 chain (4x/2x
        modes) for the small ones.
- PE:   warmup matmul (ramps the clock gate), then the chunk matmuls
        accumulating in PSUM, bias folded in via the ONES channel.

Output-DMA race (deliberate, bounded): the out DMA is issued when the f16
squares finish (s_gate), ~600ns of compute before Y is final, while the
descriptor-gen + DGE pipeline takes ~1.3us before the engines read Y.
Same pattern the previous baseline shipped (bias-add vs descriptor-gen),
with a wider margin measured in the cost model (~600ns).
"""

import numpy as np

N_IN, N_OUT, SPLINE_K, G = 128, 128, 3, 8
BATCH = 512
EDGES = N_IN * N_OUT
N_KNOTS = G + 2 * SPLINE_K + 1          # 15
N_CORES = 8
BSHARD = BATCH // N_CORES               # 64
# relu channels whose max |x-t|^3 is below this go to fp16
F16_AMP_THRESH = 1.1

_COMPILED = {}


def _beta_from_bspline(knots, c_basis):
    """Truncated-power coefficients beta (EDGES, N_KNOTS) such that
    sum_g c[e,g] B_{e,g}(x) == sum_m beta[e,m] relu(x - knots[e,m])^3
    exactly (computed in float64)."""
    E = knots.shape[0]
    t = knots.astype(np.float64)
    c = c_basis.astype(np.float64)

    def deriv(c, k):
        m = c.shape[1]
        cpad = np.concatenate(
            [np.zeros((E, 1)), c, np.zeros((E, 1))], axis=1)
        g = np.arange(m + 1)
        denom = t[:, g + k] - t[:, g]
        with np.errstate(divide="ignore", invalid="ignore"):
            d = k * (cpad[:, 1:] - cpad[:, :-1]) / denom
        return np.where(denom == 0, 0.0, d)

    c3 = deriv(deriv(deriv(c, 3), 2), 1)          # s''' per interval (E, 14)
    c3pad = np.concatenate([np.zeros((E, 1)), c3, np.zeros((E, 1))], axis=1)
    return (c3pad[:, 1:] - c3pad[:, :-1]) / 6.0   # (E, 15)


def _reference_numpy(x, knots, c_basis, c_spl, c_res, bias):
    """Exact (slow) fallback for inputs the factorized kernel can't
    handle (knot rows differing across the n_out axis)."""
    batch = x.shape[0]
    x_ext = np.broadcast_to(x[:, None, :], (batch, N_OUT, N_IN)).reshape(batch, EDGES).T
    grid = knots[:, :, None]
    x_in = x_ext[:, None, :]
    b = ((x_in >= grid[:, :-1]) & (x_in < grid[:, 1:])).astype(np.float32)
    for order in range(1, SPLINE_K + 1):
        n0 = grid[:, order:-1] - grid[:, :-(order + 1)]
        n1 = grid[:, order + 1:] - grid[:, 1:-order]
        with np.errstate(divide="ignore", invalid="ignore"):
            left = np.where(n0 == 0, 0.0, (x_in - grid[:, :-(order + 1)]) / n0)
            right = np.where(n1 == 0, 0.0, (grid[:, order + 1:] - x_in) / n1)
        b = left * b[:, :-1] + right * b[:, 1:]
    spl = np.einsum("eg,egb->eb", c_basis, b).T
    y = c_spl.reshape(1, EDGES) * spl
    sig = 1.0 / (1.0 + np.exp(-x_ext.T))
    y = y + c_res.reshape(1, EDGES) * (x_ext.T * sig)
    return (y.reshape(batch, N_OUT, N_IN).sum(axis=2) + bias).astype(np.float32)


def _build_program(nf32, nf16):
    """Raw-bacc per-core program.

    Channels: 4 fp32 poly [x, x^2, x^3, ones] + nf32 fp32 relu-cubes
    + fp16 [silu] + nf16 fp16 relu-cubes.  nf32 >= 1, nf16 >= 0.
    """
    import concourse.bass as bass
    import concourse.mybir as mybir
    from concourse import bacc
    from concourse.hw_specs import get_activation_tables

    S = BSHARD
    NCH32 = 3 + nf32                    # fp32 chunks in W32
    NCH16 = 1 + nf16                    # fp16 chunks in W16
    dt32 = mybir.dt.float32
    dt16 = mybir.dt.float16
    ACTF = mybir.ActivationFunctionType
    ALU = mybir.AluOpType

    class SlimBlock(bass.BassBlock):
        """Skip the exit drain + all-engine barrier; completion is carried
        by the explicit semaphore chain ending in s_y."""
        def __exit__(self, exc_type, exc_val, exc_tb):
            if exc_type is None:
                for engine, last_body in self.last_body.items():
                    with self.bass.body(last_body, parent=self.bass.cur_bb,
                                        allow_existing_parent=True):
                        engine.br(self.end_bb)
                self.bass.switch_bb(self.end_bb)

    class FastBacc(bacc.Bacc):
        """Skip the constructor's const-AP entry barrier: nothing reads the
        const tiles before a much-later semaphore wait."""
        _skip_entry_barrier = True

        def all_engine_barrier(self, **kw):
            if getattr(self, "_skip_entry_barrier", False):
                return
            return super().all_engine_barrier(**kw)

    # Bass.__init__ seeds four const tiles with Pool memsets.  None of
    # them needs real initialization here: the silu bias reads a zero
    # column of xp instead of the const-zero tile, and the warmup matmul
    # result is discarded.  The memsets would delay Pool's SWDGE
    # descriptor-gen by ~300ns, which delays the first W transfer.  The
    # patch goes on BassGpSimd (not BassSharedVectorInterface) so it wins
    # the MRO over the Rust base class's memset.
    _had_own = "memset" in vars(bass.BassGpSimd)
    _orig_memset = bass.BassGpSimd.memset

    def _filtered_memset(self, ap, constant):
        return None

    bass.BassGpSimd.memset = _filtered_memset
    try:
        nc = FastBacc("TRN2", target_bir_lowering=False, debug=False)
    finally:
        if _had_own:
            bass.BassGpSimd.memset = _orig_memset
        else:
            del bass.BassGpSimd.memset
    nc._skip_entry_barrier = False

    xp = nc.dram_tensor("xp", [128, 128], dt32, kind="ExternalInput")
    w32 = nc.dram_tensor("w32", [N_IN, NCH32 * N_OUT], dt32, kind="ExternalInput")
    w16 = nc.dram_tensor("w16", [N_IN, NCH16 * N_OUT], dt16, kind="ExternalInput")
    y = nc.dram_tensor("y", [N_OUT, S], dt32, kind="ExternalOutput")

    from contextlib import ExitStack
    with ExitStack() as stack:
        ent = stack.enter_context
        XP = ent(nc.sbuf_tensor([128, 128], dt32))
        W32 = ent(nc.sbuf_tensor([N_IN, NCH32 * N_OUT], dt32))
        W16 = ent(nc.sbuf_tensor([N_IN, NCH16 * N_OUT], dt16))
        PHI32 = ent(nc.sbuf_tensor([128, 2 * S], dt32))    # [x^2 | x^3]
        R32 = ent(nc.sbuf_tensor([128, max(nf32, 1) * S], dt32))
        SQ32 = ent(nc.sbuf_tensor([128, max(nf32, 1) * S], dt32))
        XH = ent(nc.sbuf_tensor([128, S], dt16))
        PHI16 = ent(nc.sbuf_tensor([128, NCH16 * S], dt16))  # [silu | cubes]
        SQ16 = ent(nc.sbuf_tensor([128, max(nf16, 1) * S], dt16))
        Y = ent(nc.sbuf_tensor([N_OUT, S], dt32))
        acc = ent(nc.psum_tensor([N_OUT, S], dt32))
        dump = ent(nc.psum_tensor([1, 512], dt32))
        s_pack = ent(nc.semaphore("s_pack"))
        s_wa = ent(nc.semaphore("s_wa"))
        s_wb = ent(nc.semaphore("s_wb"))
        s_wc = ent(nc.semaphore("s_wc"))
        s_pool = ent(nc.semaphore("s_pool"))
        s_silu = ent(nc.semaphore("s_silu"))
        s_xh = ent(nc.semaphore("s_xh"))
        s_cub1 = ent(nc.semaphore("s_cub1"))
        s_cub2 = ent(nc.semaphore("s_cub2"))
        s_mm = ent(nc.semaphore("s_mm"))
        s_cp = ent(nc.semaphore("s_cp"))
        s_y = ent(nc.semaphore("s_y"))
        block = ent(SlimBlock(nc, "main"))
        X = XP[:, :S]
        NTA = XP[:, S:S + nf32]                  # -t for fp32 relu chunks
        NTB = XP[:, S + nf32:S + nf32 + nf16]    # -t for fp16 relu chunks
        BIAS = XP[:, S + nf32 + nf16:S + nf32 + nf16 + 1]  # folded bias per o
        W32A_COLS = 3 * N_OUT                    # [x, x^2, x^3]

        @block.sync
        def _(sp):
            sp.dma_start(out=XP[:], in_=xp.ap()).then_inc(s_pack, 16)
            sp.dma_start(out=W16[:], in_=w16.ap()).then_inc(s_wc, 16)
            # Strictly after the bias-add copy lands in Y.  (A same-tick
            # race gating this on s_mm looks attractive -- the ~1.3us
            # descriptor-gen pipeline vs the ~450ns copy -- but on a cold
            # device the first-ever execution reads stale SBUF: the runtime
            # does not order a DMA read after a same-tick engine write.)
            sp.wait_ge(s_cp, 1)
            sp.dma_start(out=y.ap(), in_=Y[:]).then_inc(s_y, 16)
            sp.wait_ge(s_y, 16)

        @block.gpsimd
        def _(pool):
            pool.dma_start(out=W32[:, :W32A_COLS],
                           in_=w32.ap()[:, :W32A_COLS]).then_inc(s_wa, 16)
            pool.wait_ge(s_pack, 16)
            nc.gpsimd.tensor_mul(PHI32[:, :S], X, X)
            nc.gpsimd.tensor_mul(PHI32[:, S:2 * S], PHI32[:, :S], X) \
                .then_inc(s_pool, 1)

        @block.scalar
        def _(act):
            act.dma_start(out=W32[:, W32A_COLS:],
                          in_=w32.ap()[:, W32A_COLS:]).then_inc(s_wb, 16)
            tabs = get_activation_tables(nc.m.arch)
            set_id = list(tabs).index("silu_and_others")
            ld = mybir.InstLoadActFuncSet(
                name=nc.get_next_instruction_name(), ins=[], outs=[],
                act_func_set_id=set_id)
            ld.engine = mybir.EngineType.Activation
            nc.scalar.add_instruction(ld)
            act.wait_ge(s_pack, 16)
            nc.scalar.activation(PHI16[:, :S], X, ACTF.Silu,
                                 bias=XP[:, 127:128]).then_inc(s_silu, 1)
            nc.scalar.activation(XH[:], X, ACTF.Copy).then_inc(s_xh, 1)

        @block.vector
        def _(dve):
            dve.wait_ge(s_pack, 16)
            for j in range(nf32):
                nc.vector.tensor_scalar(
                    R32[:, j * S:(j + 1) * S], X, NTA[:, j:j + 1], 0.0,
                    op0=ALU.add, op1=ALU.max)
            g32 = nf32 * S
            nc.vector.tensor_mul(SQ32[:, :g32], R32[:, :g32], R32[:, :g32])
            nc.vector.tensor_mul(R32[:, :g32], SQ32[:, :g32], R32[:, :g32]) \
                .then_inc(s_cub1, 1)
            if nf16:
                dve.wait_ge(s_xh, 1)
                for j in range(nf16):
                    nc.vector.tensor_scalar(
                        PHI16[:, (1 + j) * S:(2 + j) * S], XH[:],
                        NTB[:, j:j + 1], 0.0, op0=ALU.add, op1=ALU.max)
                g16 = nf16 * S
                RH = PHI16[:, S:S + g16]
                nc.vector.tensor_mul(SQ16[:, :g16], RH, RH)
                nc.vector.tensor_mul(RH, SQ16[:, :g16], RH).then_inc(s_cub2, 1)
            # PSUM -> SBUF move fused with the bias add (bias is rank-1 so
            # it folds into the copy as a per-partition scalar; partitions
            # of Y are n_out, matching bias indexing)
            dve.wait_ge(s_mm, 1)
            nc.vector.tensor_scalar_add(Y[:], acc[:], BIAS).then_inc(s_cp, 1)


        @block.tensor
        def _(pe):
            # Discarded wide matmul: keeps the PE HAM clock-gate ramping
            # during the DMA wait so the real matmuls run at 2.4 GHz
            # instead of the cold 1.2 GHz (reads whatever is in W32; the
            # result is never consumed).
            zc = nc.const_aps.aps[(mybir.dt.float32, 0.0)]
            nc.tensor.matmul(dump[:], zc, W32[:, :512],
                             start=True, stop=True, skip_group_check=True)
            # Second warmup: carries the busy streak through the W-DMA wait
            # so the real matmuls run at the full 2.4GHz p-state (a ~1.8us
            # idle gap demotes them to the mid p-state, 2x slower).
            nc.tensor.matmul(dump[:, :64], zc, W32[:, :64],
                             start=True, stop=True, skip_group_check=True)
            pe.wait_ge(s_wa, 16)
            pe.wait_ge(s_pack, 16)
            pe.wait_ge(s_pool, 1)
            nc.tensor.matmul(acc[:], W32[:, :N_OUT], X, start=True, stop=False)
            nc.tensor.matmul(acc[:], W32[:, N_OUT:2 * N_OUT], PHI32[:, :S],
                             start=False, stop=False)
            nc.tensor.matmul(acc[:], W32[:, 2 * N_OUT:3 * N_OUT],
                             PHI32[:, S:2 * S], start=False, stop=False)
            pe.wait_ge(s_wb, 16)
            pe.wait_ge(s_cub1, 1)
            for j in range(nf32):
                nc.tensor.matmul(acc[:], W32[:, (3 + j) * N_OUT:(4 + j) * N_OUT],
                                 R32[:, j * S:(j + 1) * S],
                                 start=False, stop=False)
            pe.wait_ge(s_wc, 16)
            pe.wait_ge(s_silu, 1)
            mm = nc.tensor.matmul(acc[:], W16[:, :N_OUT], PHI16[:, :S],
                                  start=False, stop=(nf16 == 0))
            if nf16:
                pe.wait_ge(s_cub2, 1)
                for j in range(nf16):
                    mm = nc.tensor.matmul(
                        acc[:], W16[:, (1 + j) * N_OUT:(2 + j) * N_OUT],
                        PHI16[:, (1 + j) * S:(2 + j) * S],
                        start=False, stop=(j == nf16 - 1))
            mm.then_inc(s_mm, 1)

    nc.compile()
    return nc


def kernel(x, knots, c_basis, c_spl, c_res, bias):
    x = np.asarray(x, np.float32)
    knots = np.asarray(knots, np.float32)
    c_basis = np.asarray(c_basis, np.float32)
    c_spl = np.asarray(c_spl, np.float32)
    c_res = np.asarray(c_res, np.float32)
    bias = np.asarray(bias, np.float32)

    # Factorization requires the knot row to be shared across the n_out
    # axis for each input column i (true for the reference's broadcast
    # knots).  Otherwise fall back to the exact host implementation.
    kr = knots.reshape(N_OUT, N_IN, N_KNOTS)
    t_col = kr[0]                                     # (N_IN, N_KNOTS)
    if not np.array_equal(kr, np.broadcast_to(t_col[None], kr.shape)):
        return _reference_numpy(x, knots, c_basis, c_spl, c_res, bias)
    # Truncated powers don't vanish past the last knot (where the
    # reference's B-spline support ends), so x beyond it needs the
    # exact path.
    if np.any(x >= t_col[:, -1][None, :]):
        return _reference_numpy(x, knots, c_basis, c_spl, c_res, bias)

    beta = _beta_from_bspline(knots, c_basis)         # (EDGES, 15) f64
    xmin = x.min(axis=0)                              # (N_IN,)
    xmax = x.max(axis=0)

    # Per-edge split: knots with t <= xmin(i) never truncate -> fold into
    # the poly channels; t >= xmax(i) contribute nothing; the rest are
    # relu features, chunked by knot index m.
    ii = np.tile(np.arange(N_IN), N_OUT)
    polyM = knots <= xmin[ii][:, None]                # (EDGES, 15)
    activeM = knots < xmax[ii][:, None]
    reluM = activeM & ~polyM
    relu_cols = np.nonzero(reluM.any(axis=0))[0]

    # Poly coefficients sum_m beta*(x-t)^3 -> a0..a3 per edge (float64).
    a4 = np.zeros((EDGES, 4))
    for m in range(N_KNOTS):
        b = beta[:, m] * polyM[:, m]
        t = knots[:, m].astype(np.float64)
        a4[:, 0] += b * (-t ** 3)
        a4[:, 1] += b * (3 * t ** 2)
        a4[:, 2] += b * (-3 * t)
        a4[:, 3] += b
    cs = c_spl.reshape(EDGES).astype(np.float64)
    Wpoly = cs[:, None] * a4                          # (EDGES, 4)
    bias_fold = bias.astype(np.float64) + \
        Wpoly[:, 0].reshape(N_OUT, N_IN).sum(axis=1)  # a0 folded into bias
    Wrelu = (cs[:, None] * beta)[:, relu_cols] * reluM[:, relu_cols]

    # fp32 vs fp16 relu chunks: fp16 only where the max feature amplitude
    # (xmax - t)^3 stays small, so the ill-conditioned truncated-power
    # cancellation keeps ~fp32 headroom (measured rel err ~3e-3).
    amp = np.maximum(xmax[:, None] - t_col[:, relu_cols], 0.0) ** 3  # (N_IN, R)
    is16 = amp.max(axis=0) <= F16_AMP_THRESH
    cols32 = relu_cols[~is16]
    cols16 = relu_cols[is16]
    nf32, nf16 = len(cols32), len(cols16)
    if nf32 == 0 and len(relu_cols):
        # program requires >= 1 fp32 relu chunk; promote the largest
        cols32 = relu_cols[-1:]
        cols16 = relu_cols[:-1]
        nf32, nf16 = 1, len(cols16)
    elif nf32 == 0:
        # no relu features at all: use a dummy all-zero fp32 chunk
        cols32 = np.array([0])
        nf32 = 1
        Wrelu = np.zeros((EDGES, 1))
        relu_cols = cols32

    def chunkT(w):                                    # (EDGES,) -> (N_IN, N_OUT)
        return np.ascontiguousarray(
            w.reshape(N_OUT, N_IN).T.astype(np.float32))

    WrFull = np.zeros((EDGES, N_KNOTS))
    WrFull[:, relu_cols] = Wrelu
    w32_chunks = [chunkT(Wpoly[:, 1]), chunkT(Wpoly[:, 2]), chunkT(Wpoly[:, 3])]
    for m in cols32:
        w32_chunks.append(chunkT(WrFull[:, m]))
    w32_host = np.ascontiguousarray(
        np.concatenate(w32_chunks, axis=1), dtype=np.float32)
    w16_chunks = [np.ascontiguousarray(c_res.T.astype(np.float16))]
    for m in cols16:
        w16_chunks.append(chunkT(WrFull[:, m]).astype(np.float16))
    w16_host = np.ascontiguousarray(
        np.concatenate(w16_chunks, axis=1), dtype=np.float16)

    # xp pack: [x.T shard | -t(fp32 chunks) | -t(fp16 chunks) | bias | pad]
    nt32 = np.ascontiguousarray(-t_col[:, cols32], dtype=np.float32)
    nt16 = np.ascontiguousarray(-t_col[:, cols16], dtype=np.float32)
    xT = np.ascontiguousarray(x.T, dtype=np.float32)  # (N_IN, BATCH)
    npad = 128 - BSHARD - nf32 - nf16
    pad = np.zeros((N_IN, npad), np.float32)
    pad[:, 0] = bias_fold.astype(np.float32)  # per-partition (= per-o) bias

    key = (nf32, nf16)
    if key not in _COMPILED:
        _COMPILED[key] = _build_program(nf32, nf16)
    nc = _COMPILED[key]

    from concourse.bass_utils import run_bass_kernel_spmd
    core_ids = list(range(N_CORES))
    in_maps = []
    for c in core_ids:
        pack = np.concatenate(
            [xT[:, c * BSHARD:(c + 1) * BSHARD], nt32, nt16, pad], axis=1)
        in_maps.append({"xp": np.ascontiguousarray(pack),
                        "w32": w32_host, "w16": w16_host})
    res = run_bass_kernel_spmd(nc, in_maps, core_ids)
    y_oT = np.concatenate([res.results[c]["y"] for c in core_ids], axis=1)
    return np.ascontiguousarray(y_oT.T, dtype=np.float32)


# revision 24
# speedup vs baseline: 1.0547x; 1.0399x over previous
"""DenseKANLayer Trainium2 kernel.

Math: for each edge e=(o,i), the reference computes a cubic B-spline
s_e(x) = sum_g c_basis[e,g] * B_{e,g}(x) on the 15-point knot row of e,
then y[b,o] = sum_i c_spl[o,i]*s_(o,i)(x[b,i]) + c_res[o,i]*silu(x[b,i]) + bias[o].

A cubic B-spline combination is exactly a sum of truncated powers:
s_e(x) = sum_m beta[e,m] * relu(x - t_m)^3 (beta = jump of s'''/6 per knot,
computed host-side in float64).  Knots with t_m <= min_b x[b,i] never
truncate, so their terms are plain cubics: folded host-side into 4 "poly"
channels {1, x, x^2, x^3} (the constant one merges with bias).  Knots with
t_m >= max x contribute nothing.  What remains is a small set of true relu
features.  Folding c_spl gives one PSUM-accumulated matmul stack:

    y.T = sum_c Wc @ Phi_c,   channels c = [x, x^2, x^3, ones(=bias),
                                            relu-cubes..., silu]

Precision: the truncated-power basis is ill-conditioned (terms ~30x the
result cancel in the fp32 PSUM), so W and Phi need ~fp32 for channels with
large feature amplitude.  Channels whose max |x - t|^3 is small (knots near
the top of the data range) and the silu channel are safe in fp16 (measured
rel err ~3e-3 vs the 2e-2 budget; fp16 keeps the 2-byte DVE fast modes and
1-cycle/row PE rate).

Per-core program (batch sharded 8 ways, S=64 columns/core):
- SP:   DMA1 xp = [x.T shard | -t scalars | pad] (128x128 f32, the
        latency-critical load), DMA W16 [silu + small relu chunks] (f16),
        then the output DMA (see race note below).
- Pool: DMA W32a [x,x^2,x^3,bias] via SWDGE (own descriptor unit, so its
        gen overlaps the HWDGE gens), memset ONES, x^2/x^3 products, and
        the final PSUM->SBUF copy.
- ACT:  DMA W32b [big relu chunks], silu table load, silu (f16 out),
        x->f16 cast for the f16 relu features.
- DVE:  fp32 relu+square+cube chain for the big chunks, f16 chain (4x/2x
        modes) for the small ones.
- PE:   warmup matmul (ramps the clock gate), then the chunk matmuls
        accumulating in PSUM, bias folded in via the ONES channel.

Output-DMA race (deliberate, bounded): the out DMA is issued when the f16
squares finish (s_gate), ~600ns of compute before Y is final, while the
descriptor-gen + DGE pipeline takes ~1.3us before the engines read Y.
Same pattern the previous baseline shipped (bias-add vs descriptor-gen),
with a wider margin measured in the cost model (~600ns).
"""

import numpy as np

N_IN, N_OUT, SPLINE_K, G = 128, 128, 3, 8
BATCH = 512
EDGES = N_IN * N_OUT
N_KNOTS = G + 2 * SPLINE_K + 1          # 15
N_CORES = 8
BSHARD = BATCH // N_CORES               # 64
# relu channels whose max |x-t|^3 is below this go to fp16
F16_AMP_THRESH = 1.1

_COMPILED = {}


def _beta_from_bspline(knots, c_basis):
    """Truncated-power coefficients beta (EDGES, N_KNOTS) such that
    sum_g c[e,g] B_{e,g}(x) == sum_m beta[e,m] relu(x - knots[e,m])^3
    exactly (computed in float64)."""
    E = knots.shape[0]
    t = knots.astype(np.float64)
    c = c_basis.astype(np.float64)

    def deriv(c, k):
        m = c.shape[1]
        cpad = np.concatenate(
            [np.zeros((E, 1)), c, np.zeros((E, 1))], axis=1)
        g = np.arange(m + 1)
        denom = t[:, g + k] - t[:, g]
        with np.errstate(divide="ignore", invalid="ignore"):
            d = k * (cpad[:, 1:] - cpad[:, :-1]) / denom
        return np.where(denom == 0, 0.0, d)

    c3 = deriv(deriv(deriv(c, 3), 2), 1)          # s''' per interval (E, 14)
    c3pad = np.concatenate([np.zeros((E, 1)), c3, np.zeros((E, 1))], axis=1)
    return (c3pad[:, 1:] - c3pad[:, :-1]) / 6.0   # (E, 15)


def _reference_numpy(x, knots, c_basis, c_spl, c_res, bias):
    """Exact (slow) fallback for inputs the factorized kernel can't
    handle (knot rows differing across the n_out axis)."""
    batch = x.shape[0]
    x_ext = np.broadcast_to(x[:, None, :], (batch, N_OUT, N_IN)).reshape(batch, EDGES).T
    grid = knots[:, :, None]
    x_in = x_ext[:, None, :]
    b = ((x_in >= grid[:, :-1]) & (x_in < grid[:, 1:])).astype(np.float32)
    for order in range(1, SPLINE_K + 1):
        n0 = grid[:, order:-1] - grid[:, :-(order + 1)]
        n1 = grid[:, order + 1:] - grid[:, 1:-order]
        with np.errstate(divide="ignore", invalid="ignore"):
            left = np.where(n0 == 0, 0.0, (x_in - grid[:, :-(order + 1)]) / n0)
            right = np.where(n1 == 0, 0.0, (grid[:, order + 1:] - x_in) / n1)
        b = left * b[:, :-1] + right * b[:, 1:]
    spl = np.einsum("eg,egb->eb", c_basis, b).T
    y = c_spl.reshape(1, EDGES) * spl
    sig = 1.0 / (1.0 + np.exp(-x_ext.T))
    y = y + c_res.reshape(1, EDGES) * (x_ext.T * sig)
    return (y.reshape(batch, N_OUT, N_IN).sum(axis=2) + bias).astype(np.float32)


def _build_program(nfl, nfr):
    """Raw-bacc per-core program.

    Channels: 3 fp32 poly [x, x^2, x^3] + fp16 [silu] + nfl mirrored
    relu-cubes relu(t-x)^3 + nfr plain relu-cubes relu(x-t)^3 (all fp16;
    the mirror identity relu(x-t)^3 = (x-t)^3 + relu(t-x)^3 keeps every
    relu feature's amplitude small enough for fp16).  nfl, nfr >= 0,
    nfl + nfr >= 1.
    """
    import concourse.bass as bass
    import concourse.mybir as mybir
    from concourse import bacc
    from concourse.hw_specs import get_activation_tables

    S = BSHARD
    NCH32 = 3                           # fp32 chunks in W32: x, x^2, x^3
    NCHA = 1 + nfl                      # W16a: silu + mirrored relu chunks
    NCHB = nfr                          # W16b: plain relu chunks
    dt32 = mybir.dt.float32
    dt16 = mybir.dt.float16
    ACTF = mybir.ActivationFunctionType
    ALU = mybir.AluOpType

    class SlimBlock(bass.BassBlock):
        """Skip the exit drain + all-engine barrier; completion is carried
        by the explicit semaphore chain ending in s_y."""
        def __exit__(self, exc_type, exc_val, exc_tb):
            if exc_type is None:
                for engine, last_body in self.last_body.items():
                    with self.bass.body(last_body, parent=self.bass.cur_bb,
                                        allow_existing_parent=True):
                        engine.br(self.end_bb)
                self.bass.switch_bb(self.end_bb)

    class FastBacc(bacc.Bacc):
        """Skip the constructor's const-AP entry barrier: nothing reads the
        const tiles before a much-later semaphore wait."""
        _skip_entry_barrier = True

        def all_engine_barrier(self, **kw):
            if getattr(self, "_skip_entry_barrier", False):
                return
            return super().all_engine_barrier(**kw)

    # Bass.__init__ seeds four const tiles with Pool memsets.  None of
    # them needs real initialization here: the silu bias reads a zero
    # column of xp instead of the const-zero tile, and the warmup matmul
    # result is discarded.  The memsets would delay Pool's SWDGE
    # descriptor-gen by ~300ns, which delays the first W transfer.  The
    # patch goes on BassGpSimd (not BassSharedVectorInterface) so it wins
    # the MRO over the Rust base class's memset.
    _had_own = "memset" in vars(bass.BassGpSimd)
    _orig_memset = bass.BassGpSimd.memset

    def _filtered_memset(self, ap, constant):
        return None

    bass.BassGpSimd.memset = _filtered_memset
    try:
        nc = FastBacc("TRN2", target_bir_lowering=False, debug=False)
    finally:
        if _had_own:
            bass.BassGpSimd.memset = _orig_memset
        else:
            del bass.BassGpSimd.memset
    nc._skip_entry_barrier = False

    xp = nc.dram_tensor("xp", [128, 128], dt32, kind="ExternalInput")
    w32 = nc.dram_tensor("w32", [N_IN, NCH32 * N_OUT], dt32, kind="ExternalInput")
    w16a = nc.dram_tensor("w16a", [N_IN, NCHA * N_OUT], dt16, kind="ExternalInput")
    w16b = nc.dram_tensor("w16b", [N_IN, max(NCHB, 1) * N_OUT], dt16,
                          kind="ExternalInput")
    y = nc.dram_tensor("y", [N_OUT, S], dt32, kind="ExternalOutput")

    from contextlib import ExitStack
    with ExitStack() as stack:
        ent = stack.enter_context
        XP = ent(nc.sbuf_tensor([128, 128], dt32))
        W32 = ent(nc.sbuf_tensor([N_IN, NCH32 * N_OUT], dt32))
        W16A = ent(nc.sbuf_tensor([N_IN, NCHA * N_OUT], dt16))
        W16B = ent(nc.sbuf_tensor([N_IN, max(NCHB, 1) * N_OUT], dt16))
        PHI32 = ent(nc.sbuf_tensor([128, 2 * S], dt32))    # [x^2 | x^3]
        XH = ent(nc.sbuf_tensor([128, S], dt16))           # f16(x)
        XN = ent(nc.sbuf_tensor([128, S], dt16))           # f16(-x)
        PHI16 = ent(nc.sbuf_tensor([128, (1 + nfl + nfr) * S], dt16))
        SQ16 = ent(nc.sbuf_tensor([128, max(nfl + nfr, 1) * S], dt16))
        Y = ent(nc.sbuf_tensor([N_OUT, S], dt32))
        acc = ent(nc.psum_tensor([N_OUT, S], dt32))
        dump = ent(nc.psum_tensor([1, 512], dt32))
        s_pack = ent(nc.semaphore("s_pack"))
        s_wa = ent(nc.semaphore("s_wa"))
        s_wb = ent(nc.semaphore("s_wb"))
        s_wc = ent(nc.semaphore("s_wc"))
        s_pool = ent(nc.semaphore("s_pool"))
        s_silu = ent(nc.semaphore("s_silu"))
        s_cub1 = ent(nc.semaphore("s_cub1"))
        s_cub2 = ent(nc.semaphore("s_cub2"))
        s_mm = ent(nc.semaphore("s_mm"))
        s_cp = ent(nc.semaphore("s_cp"))
        s_y = ent(nc.semaphore("s_y"))
        block = ent(SlimBlock(nc, "main"))
        X = XP[:, :S]
        NTL = XP[:, S:S + nfl]                   # +t for mirrored chunks
        NTR = XP[:, S + nfl:S + nfl + nfr]       # -t for plain chunks
        BIAS = XP[:, S + nfl + nfr:S + nfl + nfr + 1]  # folded bias per o

        @block.sync
        def _(sp):
            sp.dma_start(out=XP[:], in_=xp.ap()).then_inc(s_pack, 16)
            if NCHB:
                sp.dma_start(out=W16B[:], in_=w16b.ap()).then_inc(s_wc, 16)
            # Strictly after the bias-add copy lands in Y.  (A same-tick
            # race gating this on s_mm looks attractive -- the ~1.3us
            # descriptor-gen pipeline vs the ~450ns copy -- but on a cold
            # device the first-ever execution reads stale SBUF: the runtime
            # does not order a DMA read after a same-tick engine write.)
            sp.wait_ge(s_cp, 1)
            sp.dma_start(out=y.ap(), in_=Y[:]).then_inc(s_y, 16)
            sp.wait_ge(s_y, 16)

        @block.gpsimd
        def _(pool):
            pool.dma_start(out=W32[:], in_=w32.ap()).then_inc(s_wa, 16)
            pool.wait_ge(s_pack, 16)
            nc.gpsimd.tensor_mul(PHI32[:, :S], X, X)
            nc.gpsimd.tensor_mul(PHI32[:, S:2 * S], PHI32[:, :S], X) \
                .then_inc(s_pool, 1)

        @block.scalar
        def _(act):
            act.dma_start(out=W16A[:], in_=w16a.ap()).then_inc(s_wb, 16)
            tabs = get_activation_tables(nc.m.arch)
            set_id = list(tabs).index("silu_and_others")
            ld = mybir.InstLoadActFuncSet(
                name=nc.get_next_instruction_name(), ins=[], outs=[],
                act_func_set_id=set_id)
            ld.engine = mybir.EngineType.Activation
            nc.scalar.add_instruction(ld)
            act.wait_ge(s_pack, 16)
            nc.scalar.activation(PHI16[:, :S], X, ACTF.Silu,
                                 bias=XP[:, 127:128]).then_inc(s_silu, 1)

        @block.vector
        def _(dve):
            dve.wait_ge(s_pack, 16)
            nc.vector.tensor_scalar_mul(XH[:], X, 1.0)        # f32 -> f16
            if nfl:
                # note: f16-input + immediate-scalar ops miscompute on this
                # runtime (f16 imm encoding); read the f32 X instead
                nc.vector.tensor_scalar_mul(XN[:], X, -1.0)
                for j in range(nfl):
                    nc.vector.tensor_scalar(
                        PHI16[:, (1 + j) * S:(2 + j) * S], XN[:],
                        NTL[:, j:j + 1], 0.0, op0=ALU.add, op1=ALU.max)
                gl = nfl * S
                RL = PHI16[:, S:S + gl]
                nc.vector.tensor_mul(SQ16[:, :gl], RL, RL)
                nc.vector.tensor_mul(RL, SQ16[:, :gl], RL).then_inc(s_cub1, 1)
            if nfr:
                for j in range(nfr):
                    nc.vector.tensor_scalar(
                        PHI16[:, (1 + nfl + j) * S:(2 + nfl + j) * S], XH[:],
                        NTR[:, j:j + 1], 0.0, op0=ALU.add, op1=ALU.max)
                gr = nfr * S
                RR = PHI16[:, (1 + nfl) * S:(1 + nfl) * S + gr]
                nc.vector.tensor_mul(SQ16[:, :gr], RR, RR)
                nc.vector.tensor_mul(RR, SQ16[:, :gr], RR).then_inc(s_cub2, 1)
            # PSUM -> SBUF move fused with the bias add (bias is rank-1 so
            # it folds into the copy as a per-partition scalar; partitions
            # of Y are n_out, matching bias indexing)
            dve.wait_ge(s_mm, 1)
            nc.vector.tensor_scalar_add(Y[:], acc[:], BIAS).then_inc(s_cp, 1)


        @block.tensor
        def _(pe):
            # Discarded wide matmul: keeps the PE HAM clock-gate ramping
            # during the DMA wait so the real matmuls run at 2.4 GHz
            # instead of the cold 1.2 GHz (reads whatever is in W32; the
            # result is never consumed).
            zc = nc.const_aps.aps[(mybir.dt.float32, 0.0)]
            nc.tensor.matmul(dump[:, :384], zc, W32[:, :384],
                             start=True, stop=True, skip_group_check=True)
            # Second warmup: carries the busy streak through the W-DMA wait
            # so the real matmuls run at the full 2.4GHz p-state (a ~1.8us
            # idle gap demotes them to the mid p-state, 2x slower).
            nc.tensor.matmul(dump[:, :64], zc, W32[:, :64],
                             start=True, stop=True, skip_group_check=True)
            pe.wait_ge(s_wa, 16)
            pe.wait_ge(s_pack, 16)
            pe.wait_ge(s_pool, 1)
            nc.tensor.matmul(acc[:], W32[:, :N_OUT], X, start=True, stop=False)
            nc.tensor.matmul(acc[:], W32[:, N_OUT:2 * N_OUT], PHI32[:, :S],
                             start=False, stop=False)
            nc.tensor.matmul(acc[:], W32[:, 2 * N_OUT:3 * N_OUT],
                             PHI32[:, S:2 * S], start=False, stop=False)
            pe.wait_ge(s_wb, 16)
            pe.wait_ge(s_silu, 1)
            if nfl:
                pe.wait_ge(s_cub1, 1)
            mm = nc.tensor.matmul(acc[:], W16A[:, :N_OUT], PHI16[:, :S],
                                  start=False, stop=(nfl + nfr == 0))
            for j in range(nfl):
                mm = nc.tensor.matmul(
                    acc[:], W16A[:, (1 + j) * N_OUT:(2 + j) * N_OUT],
                    PHI16[:, (1 + j) * S:(2 + j) * S],
                    start=False, stop=(nfr == 0 and j == nfl - 1))
            if nfr:
                pe.wait_ge(s_wc, 16)
                pe.wait_ge(s_cub2, 1)
                for j in range(nfr):
                    mm = nc.tensor.matmul(
                        acc[:], W16B[:, j * N_OUT:(j + 1) * N_OUT],
                        PHI16[:, (1 + nfl + j) * S:(2 + nfl + j) * S],
                        start=False, stop=(j == nfr - 1))
            mm.then_inc(s_mm, 1)

    nc.compile()
    return nc


def kernel(x, knots, c_basis, c_spl, c_res, bias):
    x = np.asarray(x, np.float32)
    knots = np.asarray(knots, np.float32)
    c_basis = np.asarray(c_basis, np.float32)
    c_spl = np.asarray(c_spl, np.float32)
    c_res = np.asarray(c_res, np.float32)
    bias = np.asarray(bias, np.float32)

    # Factorization requires the knot row to be shared across the n_out
    # axis for each input column i (true for the reference's broadcast
    # knots).  Otherwise fall back to the exact host implementation.
    kr = knots.reshape(N_OUT, N_IN, N_KNOTS)
    t_col = kr[0]                                     # (N_IN, N_KNOTS)
    if not np.array_equal(kr, np.broadcast_to(t_col[None], kr.shape)):
        return _reference_numpy(x, knots, c_basis, c_spl, c_res, bias)
    # Truncated powers don't vanish past the last knot (where the
    # reference's B-spline support ends), so x beyond it needs the
    # exact path.
    if np.any(x >= t_col[:, -1][None, :]):
        return _reference_numpy(x, knots, c_basis, c_spl, c_res, bias)

    beta = _beta_from_bspline(knots, c_basis)         # (EDGES, 15) f64
    xmin = x.min(axis=0)                              # (N_IN,)
    xmax = x.max(axis=0)

    # Per-edge split: knots with t <= xmin(i) never truncate -> fold into
    # the poly channels; t >= xmax(i) contribute nothing; the rest are
    # relu features, chunked by knot index m.
    ii = np.tile(np.arange(N_IN), N_OUT)
    polyM = knots <= xmin[ii][:, None]                # (EDGES, 15)
    activeM = knots < xmax[ii][:, None]
    reluM = activeM & ~polyM
    relu_cols = np.nonzero(reluM.any(axis=0))[0]

    # Mirror identity relu(x-t)^3 = (x-t)^3 + relu(t-x)^3: flip each relu
    # feature to whichever side has the smaller amplitude, so every chunk
    # is fp16-safe (measured rel err ~3.8e-3 vs the 2e-2 budget).  The
    # (x-t)^3 of each mirrored knot folds into the poly channels.
    amp_unm = np.maximum(xmax[:, None] - t_col[:, relu_cols], 0.0) ** 3
    amp_mir = np.maximum(t_col[:, relu_cols] - xmin[:, None], 0.0) ** 3
    mirror = amp_mir.max(axis=0) < amp_unm.max(axis=0)
    cols_l = relu_cols[mirror]                        # mirrored chunks
    cols_r = relu_cols[~mirror]                       # plain chunks
    nfl, nfr = len(cols_l), len(cols_r)

    # Poly coefficients: original poly knots plus the expanded cubic of
    # every mirrored knot (float64).
    a4 = np.zeros((EDGES, 4))

    def fold_cubic(b, t):
        a4[:, 0] += b * (-t ** 3)
        a4[:, 1] += b * (3 * t ** 2)
        a4[:, 2] += b * (-3 * t)
        a4[:, 3] += b

    for m in range(N_KNOTS):
        fold_cubic(beta[:, m] * polyM[:, m], knots[:, m].astype(np.float64))
    for m in cols_l:
        fold_cubic(beta[:, m] * reluM[:, m], knots[:, m].astype(np.float64))
    cs = c_spl.reshape(EDGES).astype(np.float64)
    Wpoly = cs[:, None] * a4                          # (EDGES, 4)
    bias_fold = bias.astype(np.float64) + \
        Wpoly[:, 0].reshape(N_OUT, N_IN).sum(axis=1)  # a0 folded into bias
    WrFull = (cs[:, None] * beta) * reluM             # (EDGES, 15)

    def chunkT(w):                                    # (EDGES,) -> (N_IN, N_OUT)
        return np.ascontiguousarray(
            w.reshape(N_OUT, N_IN).T.astype(np.float32))

    w32_host = np.ascontiguousarray(np.concatenate(
        [chunkT(Wpoly[:, 1]), chunkT(Wpoly[:, 2]), chunkT(Wpoly[:, 3])],
        axis=1), dtype=np.float32)
    w16a_chunks = [np.ascontiguousarray(c_res.T.astype(np.float16))]
    for m in cols_l:
        w16a_chunks.append(chunkT(WrFull[:, m]).astype(np.float16))
    w16a_host = np.ascontiguousarray(
        np.concatenate(w16a_chunks, axis=1), dtype=np.float16)
    if nfr:
        w16b_host = np.ascontiguousarray(np.concatenate(
            [chunkT(WrFull[:, m]).astype(np.float16) for m in cols_r],
            axis=1), dtype=np.float16)
    else:
        w16b_host = np.zeros((N_IN, N_OUT), np.float16)

    # xp pack: [x.T shard | +t(mirrored) | -t(plain) | bias | zero pad]
    ntl = np.ascontiguousarray(t_col[:, cols_l], dtype=np.float32)
    ntr = np.ascontiguousarray(-t_col[:, cols_r], dtype=np.float32)
    xT = np.ascontiguousarray(x.T, dtype=np.float32)  # (N_IN, BATCH)
    npad = 128 - BSHARD - nfl - nfr
    pad = np.zeros((N_IN, npad), np.float32)
    pad[:, 0] = bias_fold.astype(np.float32)  # per-partition (= per-o) bias

    key = (nfl, nfr)
    if key not in _COMPILED:
        _COMPILED[key] = _build_program(nfl, nfr)
    nc = _COMPILED[key]

    from concourse.bass_utils import run_bass_kernel_spmd
    core_ids = list(range(N_CORES))
    in_maps = []
    for c in core_ids:
        pack = np.concatenate(
            [xT[:, c * BSHARD:(c + 1) * BSHARD], ntl, ntr, pad], axis=1)
        in_maps.append({"xp": np.ascontiguousarray(pack),
                        "w32": w32_host, "w16a": w16a_host,
                        "w16b": w16b_host})
    res = run_bass_kernel_spmd(nc, in_maps, core_ids)
    y_oT = np.concatenate([res.results[c]["y"] for c in core_ids], axis=1)
    return np.ascontiguousarray(y_oT.T, dtype=np.float32)


# revision 25
# speedup vs baseline: 1.0685x; 1.0131x over previous
"""DenseKANLayer Trainium2 kernel.

Math: for each edge e=(o,i), the reference computes a cubic B-spline
s_e(x) = sum_g c_basis[e,g] * B_{e,g}(x) on the 15-point knot row of e,
then y[b,o] = sum_i c_spl[o,i]*s_(o,i)(x[b,i]) + c_res[o,i]*silu(x[b,i]) + bias[o].

A cubic B-spline combination is exactly a sum of truncated powers:
s_e(x) = sum_m beta[e,m] * relu(x - t_m)^3 (beta = jump of s'''/6 per knot,
computed host-side in float64).  Knots with t_m <= min_b x[b,i] never
truncate, so their terms are plain cubics: folded host-side into 4 "poly"
channels {1, x, x^2, x^3} (the constant one merges with bias).  Knots with
t_m >= max x contribute nothing.  What remains is a small set of true relu
features.  Folding c_spl gives one PSUM-accumulated matmul stack:

    y.T = sum_c Wc @ Phi_c,   channels c = [x, x^2, x^3, ones(=bias),
                                            relu-cubes..., silu]

Precision: the truncated-power basis is ill-conditioned (terms ~30x the
result cancel in the fp32 PSUM), so W and Phi need ~fp32 for channels with
large feature amplitude.  Channels whose max |x - t|^3 is small (knots near
the top of the data range) and the silu channel are safe in fp16 (measured
rel err ~3e-3 vs the 2e-2 budget; fp16 keeps the 2-byte DVE fast modes and
1-cycle/row PE rate).

Per-core program (batch sharded 8 ways, S=64 columns/core):
- SP:   DMA1 xp = [x.T shard | -t scalars | pad] (128x128 f32, the
        latency-critical load), DMA W16 [silu + small relu chunks] (f16),
        then the output DMA (see race note below).
- Pool: DMA W32a [x,x^2,x^3,bias] via SWDGE (own descriptor unit, so its
        gen overlaps the HWDGE gens), memset ONES, x^2/x^3 products, and
        the final PSUM->SBUF copy.
- ACT:  DMA W32b [big relu chunks], silu table load, silu (f16 out),
        x->f16 cast for the f16 relu features.
- DVE:  fp32 relu+square+cube chain for the big chunks, f16 chain (4x/2x
        modes) for the small ones.
- PE:   warmup matmul (ramps the clock gate), then the chunk matmuls
        accumulating in PSUM, bias folded in via the ONES channel.

Output-DMA race (deliberate, bounded): the out DMA is issued when the f16
squares finish (s_gate), ~600ns of compute before Y is final, while the
descriptor-gen + DGE pipeline takes ~1.3us before the engines read Y.
Same pattern the previous baseline shipped (bias-add vs descriptor-gen),
with a wider margin measured in the cost model (~600ns).
"""

import numpy as np

N_IN, N_OUT, SPLINE_K, G = 128, 128, 3, 8
BATCH = 512
EDGES = N_IN * N_OUT
N_KNOTS = G + 2 * SPLINE_K + 1          # 15
N_CORES = 8
BSHARD = BATCH // N_CORES               # 64
# relu channels whose max |x-t|^3 is below this go to fp16
F16_AMP_THRESH = 1.1

_COMPILED = {}


def _beta_from_bspline(knots, c_basis):
    """Truncated-power coefficients beta (EDGES, N_KNOTS) such that
    sum_g c[e,g] B_{e,g}(x) == sum_m beta[e,m] relu(x - knots[e,m])^3
    exactly (computed in float64)."""
    E = knots.shape[0]
    t = knots.astype(np.float64)
    c = c_basis.astype(np.float64)

    def deriv(c, k):
        m = c.shape[1]
        cpad = np.concatenate(
            [np.zeros((E, 1)), c, np.zeros((E, 1))], axis=1)
        g = np.arange(m + 1)
        denom = t[:, g + k] - t[:, g]
        with np.errstate(divide="ignore", invalid="ignore"):
            d = k * (cpad[:, 1:] - cpad[:, :-1]) / denom
        return np.where(denom == 0, 0.0, d)

    c3 = deriv(deriv(deriv(c, 3), 2), 1)          # s''' per interval (E, 14)
    c3pad = np.concatenate([np.zeros((E, 1)), c3, np.zeros((E, 1))], axis=1)
    return (c3pad[:, 1:] - c3pad[:, :-1]) / 6.0   # (E, 15)


def _reference_numpy(x, knots, c_basis, c_spl, c_res, bias):
    """Exact (slow) fallback for inputs the factorized kernel can't
    handle (knot rows differing across the n_out axis)."""
    batch = x.shape[0]
    x_ext = np.broadcast_to(x[:, None, :], (batch, N_OUT, N_IN)).reshape(batch, EDGES).T
    grid = knots[:, :, None]
    x_in = x_ext[:, None, :]
    b = ((x_in >= grid[:, :-1]) & (x_in < grid[:, 1:])).astype(np.float32)
    for order in range(1, SPLINE_K + 1):
        n0 = grid[:, order:-1] - grid[:, :-(order + 1)]
        n1 = grid[:, order + 1:] - grid[:, 1:-order]
        with np.errstate(divide="ignore", invalid="ignore"):
            left = np.where(n0 == 0, 0.0, (x_in - grid[:, :-(order + 1)]) / n0)
            right = np.where(n1 == 0, 0.0, (grid[:, order + 1:] - x_in) / n1)
        b = left * b[:, :-1] + right * b[:, 1:]
    spl = np.einsum("eg,egb->eb", c_basis, b).T
    y = c_spl.reshape(1, EDGES) * spl
    sig = 1.0 / (1.0 + np.exp(-x_ext.T))
    y = y + c_res.reshape(1, EDGES) * (x_ext.T * sig)
    return (y.reshape(batch, N_OUT, N_IN).sum(axis=2) + bias).astype(np.float32)


def _build_program(nfl, nfr):
    """Raw-bacc per-core program.

    Channels: 3 fp32 poly [x, x^2, x^3] + fp16 [silu] + nfl mirrored
    relu-cubes relu(t-x)^3 + nfr plain relu-cubes relu(x-t)^3 (all fp16;
    the mirror identity relu(x-t)^3 = (x-t)^3 + relu(t-x)^3 keeps every
    relu feature's amplitude small enough for fp16).  nfl, nfr >= 0,
    nfl + nfr >= 1.
    """
    import concourse.bass as bass
    import concourse.mybir as mybir
    from concourse import bacc
    from concourse.hw_specs import get_activation_tables

    S = BSHARD
    NCH32 = 3                           # fp32 chunks in W32: x, x^2, x^3
    NCHA = 1 + nfl                      # W16a: silu + mirrored relu chunks
    NCHB = nfr                          # W16b: plain relu chunks
    dt32 = mybir.dt.float32
    dt16 = mybir.dt.float16
    ACTF = mybir.ActivationFunctionType
    ALU = mybir.AluOpType

    class SlimBlock(bass.BassBlock):
        """Skip the exit drain + all-engine barrier; completion is carried
        by the explicit semaphore chain ending in s_y."""
        def __exit__(self, exc_type, exc_val, exc_tb):
            if exc_type is None:
                for engine, last_body in self.last_body.items():
                    with self.bass.body(last_body, parent=self.bass.cur_bb,
                                        allow_existing_parent=True):
                        engine.br(self.end_bb)
                self.bass.switch_bb(self.end_bb)

    class FastBacc(bacc.Bacc):
        """Skip the constructor's const-AP entry barrier: nothing reads the
        const tiles before a much-later semaphore wait."""
        _skip_entry_barrier = True

        def all_engine_barrier(self, **kw):
            if getattr(self, "_skip_entry_barrier", False):
                return
            return super().all_engine_barrier(**kw)

    # Bass.__init__ seeds four const tiles with Pool memsets.  None of
    # them needs real initialization here: the silu bias reads a zero
    # column of xp instead of the const-zero tile, and the warmup matmul
    # result is discarded.  The memsets would delay Pool's SWDGE
    # descriptor-gen by ~300ns, which delays the first W transfer.  The
    # patch goes on BassGpSimd (not BassSharedVectorInterface) so it wins
    # the MRO over the Rust base class's memset.
    _had_own = "memset" in vars(bass.BassGpSimd)
    _orig_memset = bass.BassGpSimd.memset

    def _filtered_memset(self, ap, constant):
        return None

    bass.BassGpSimd.memset = _filtered_memset
    try:
        nc = FastBacc("TRN2", target_bir_lowering=False, debug=False)
    finally:
        if _had_own:
            bass.BassGpSimd.memset = _orig_memset
        else:
            del bass.BassGpSimd.memset
    nc._skip_entry_barrier = False

    xp = nc.dram_tensor("xp", [128, 128], dt32, kind="ExternalInput")
    w32 = nc.dram_tensor("w32", [N_IN, NCH32 * N_OUT], dt32, kind="ExternalInput")
    w16a = nc.dram_tensor("w16a", [N_IN, NCHA * N_OUT], dt16, kind="ExternalInput")
    w16b = nc.dram_tensor("w16b", [N_IN, max(NCHB, 1) * N_OUT], dt16,
                          kind="ExternalInput")
    y = nc.dram_tensor("y", [N_OUT, S], dt16, kind="ExternalOutput")

    from contextlib import ExitStack
    with ExitStack() as stack:
        ent = stack.enter_context
        XP = ent(nc.sbuf_tensor([128, 128], dt32))
        W32 = ent(nc.sbuf_tensor([N_IN, NCH32 * N_OUT], dt32))
        W16A = ent(nc.sbuf_tensor([N_IN, NCHA * N_OUT], dt16))
        W16B = ent(nc.sbuf_tensor([N_IN, max(NCHB, 1) * N_OUT], dt16))
        PHI32 = ent(nc.sbuf_tensor([128, 2 * S], dt32))    # [x^2 | x^3]
        XH = ent(nc.sbuf_tensor([128, S], dt16))           # f16(x)
        XN = ent(nc.sbuf_tensor([128, S], dt16))           # f16(-x)
        PHI16 = ent(nc.sbuf_tensor([128, (1 + nfl + nfr) * S], dt16))
        SQ16 = ent(nc.sbuf_tensor([128, max(nfl + nfr, 1) * S], dt16))
        Y = ent(nc.sbuf_tensor([N_OUT, S], dt16))
        acc = ent(nc.psum_tensor([N_OUT, S], dt32))
        dump = ent(nc.psum_tensor([1, 512], dt32))
        s_pack = ent(nc.semaphore("s_pack"))
        s_wa = ent(nc.semaphore("s_wa"))
        s_wb = ent(nc.semaphore("s_wb"))
        s_wc = ent(nc.semaphore("s_wc"))
        s_pool = ent(nc.semaphore("s_pool"))
        s_silu = ent(nc.semaphore("s_silu"))
        s_cub1 = ent(nc.semaphore("s_cub1"))
        s_cub2 = ent(nc.semaphore("s_cub2"))
        s_mm = ent(nc.semaphore("s_mm"))
        s_cp = ent(nc.semaphore("s_cp"))
        s_y = ent(nc.semaphore("s_y"))
        block = ent(SlimBlock(nc, "main"))
        X = XP[:, :S]
        NTL = XP[:, S:S + nfl]                   # +t for mirrored chunks
        NTR = XP[:, S + nfl:S + nfl + nfr]       # -t for plain chunks
        BIAS = XP[:, S + nfl + nfr:S + nfl + nfr + 1]  # folded bias per o

        @block.sync
        def _(sp):
            sp.dma_start(out=XP[:], in_=xp.ap()).then_inc(s_pack, 16)
            if NCHB:
                sp.dma_start(out=W16B[:], in_=w16b.ap()).then_inc(s_wc, 16)
            # Strictly after the bias-add copy lands in Y.  (A same-tick
            # race gating this on s_mm looks attractive -- the ~1.3us
            # descriptor-gen pipeline vs the ~450ns copy -- but on a cold
            # device the first-ever execution reads stale SBUF: the runtime
            # does not order a DMA read after a same-tick engine write.)
            sp.wait_ge(s_cp, 1)
            sp.dma_start(out=y.ap(), in_=Y[:]).then_inc(s_y, 16)
            sp.wait_ge(s_y, 16)

        @block.gpsimd
        def _(pool):
            pool.dma_start(out=W32[:], in_=w32.ap()).then_inc(s_wa, 16)
            pool.wait_ge(s_pack, 16)
            nc.gpsimd.tensor_mul(PHI32[:, :S], X, X)
            nc.gpsimd.tensor_mul(PHI32[:, S:2 * S], PHI32[:, :S], X) \
                .then_inc(s_pool, 1)

        @block.scalar
        def _(act):
            act.dma_start(out=W16A[:], in_=w16a.ap()).then_inc(s_wb, 16)
            tabs = get_activation_tables(nc.m.arch)
            set_id = list(tabs).index("silu_and_others")
            ld = mybir.InstLoadActFuncSet(
                name=nc.get_next_instruction_name(), ins=[], outs=[],
                act_func_set_id=set_id)
            ld.engine = mybir.EngineType.Activation
            nc.scalar.add_instruction(ld)
            act.wait_ge(s_pack, 16)
            nc.scalar.activation(PHI16[:, :S], X, ACTF.Silu,
                                 bias=XP[:, 127:128]).then_inc(s_silu, 1)

        @block.vector
        def _(dve):
            dve.wait_ge(s_pack, 16)
            nc.vector.tensor_scalar_mul(XH[:], X, 1.0)        # f32 -> f16
            if nfl:
                # note: f16-input + immediate-scalar ops miscompute on this
                # runtime (f16 imm encoding); read the f32 X instead
                nc.vector.tensor_scalar_mul(XN[:], X, -1.0)
                for j in range(nfl):
                    nc.vector.tensor_scalar(
                        PHI16[:, (1 + j) * S:(2 + j) * S], XN[:],
                        NTL[:, j:j + 1], 0.0, op0=ALU.add, op1=ALU.max)
                gl = nfl * S
                RL = PHI16[:, S:S + gl]
                nc.vector.tensor_mul(SQ16[:, :gl], RL, RL)
                nc.vector.tensor_mul(RL, SQ16[:, :gl], RL).then_inc(s_cub1, 1)
            if nfr:
                for j in range(nfr):
                    nc.vector.tensor_scalar(
                        PHI16[:, (1 + nfl + j) * S:(2 + nfl + j) * S], XH[:],
                        NTR[:, j:j + 1], 0.0, op0=ALU.add, op1=ALU.max)
                gr = nfr * S
                RR = PHI16[:, (1 + nfl) * S:(1 + nfl) * S + gr]
                nc.vector.tensor_mul(SQ16[:, :gr], RR, RR)
                nc.vector.tensor_mul(RR, SQ16[:, :gr], RR).then_inc(s_cub2, 1)
            # PSUM -> SBUF move fused with the bias add (bias is rank-1 so
            # it folds into the copy as a per-partition scalar; partitions
            # of Y are n_out, matching bias indexing)
            dve.wait_ge(s_mm, 1)
            nc.vector.tensor_scalar_add(Y[:], acc[:], BIAS).then_inc(s_cp, 1)


        @block.tensor
        def _(pe):
            # Discarded wide matmul: keeps the PE HAM clock-gate ramping
            # during the DMA wait so the real matmuls run at 2.4 GHz
            # instead of the cold 1.2 GHz (reads whatever is in W32; the
            # result is never consumed).
            zc = nc.const_aps.aps[(mybir.dt.float32, 0.0)]
            nc.tensor.matmul(dump[:, :384], zc, W32[:, :384],
                             start=True, stop=True, skip_group_check=True)
            # Second warmup: carries the busy streak through the W-DMA wait
            # so the real matmuls run at the full 2.4GHz p-state (a ~1.8us
            # idle gap demotes them to the mid p-state, 2x slower).
            nc.tensor.matmul(dump[:, :64], zc, W32[:, :64],
                             start=True, stop=True, skip_group_check=True)
            pe.wait_ge(s_wa, 16)
            pe.wait_ge(s_pack, 16)
            pe.wait_ge(s_pool, 1)
            nc.tensor.matmul(acc[:], W32[:, :N_OUT], X, start=True, stop=False)
            nc.tensor.matmul(acc[:], W32[:, N_OUT:2 * N_OUT], PHI32[:, :S],
                             start=False, stop=False)
            nc.tensor.matmul(acc[:], W32[:, 2 * N_OUT:3 * N_OUT],
                             PHI32[:, S:2 * S], start=False, stop=False)
            pe.wait_ge(s_wb, 16)
            pe.wait_ge(s_silu, 1)
            if nfl:
                pe.wait_ge(s_cub1, 1)
            mm = nc.tensor.matmul(acc[:], W16A[:, :N_OUT], PHI16[:, :S],
                                  start=False, stop=(nfl + nfr == 0))
            for j in range(nfl):
                mm = nc.tensor.matmul(
                    acc[:], W16A[:, (1 + j) * N_OUT:(2 + j) * N_OUT],
                    PHI16[:, (1 + j) * S:(2 + j) * S],
                    start=False, stop=(nfr == 0 and j == nfl - 1))
            if nfr:
                pe.wait_ge(s_wc, 16)
                pe.wait_ge(s_cub2, 1)
                for j in range(nfr):
                    mm = nc.tensor.matmul(
                        acc[:], W16B[:, j * N_OUT:(j + 1) * N_OUT],
                        PHI16[:, (1 + nfl + j) * S:(2 + nfl + j) * S],
                        start=False, stop=(j == nfr - 1))
            mm.then_inc(s_mm, 1)

    nc.compile()
    return nc


def kernel(x, knots, c_basis, c_spl, c_res, bias):
    x = np.asarray(x, np.float32)
    knots = np.asarray(knots, np.float32)
    c_basis = np.asarray(c_basis, np.float32)
    c_spl = np.asarray(c_spl, np.float32)
    c_res = np.asarray(c_res, np.float32)
    bias = np.asarray(bias, np.float32)

    # Factorization requires the knot row to be shared across the n_out
    # axis for each input column i (true for the reference's broadcast
    # knots).  Otherwise fall back to the exact host implementation.
    kr = knots.reshape(N_OUT, N_IN, N_KNOTS)
    t_col = kr[0]                                     # (N_IN, N_KNOTS)
    if not np.array_equal(kr, np.broadcast_to(t_col[None], kr.shape)):
        return _reference_numpy(x, knots, c_basis, c_spl, c_res, bias)
    # Truncated powers don't vanish past the last knot (where the
    # reference's B-spline support ends), so x beyond it needs the
    # exact path.
    if np.any(x >= t_col[:, -1][None, :]):
        return _reference_numpy(x, knots, c_basis, c_spl, c_res, bias)

    beta = _beta_from_bspline(knots, c_basis)         # (EDGES, 15) f64
    xmin = x.min(axis=0)                              # (N_IN,)
    xmax = x.max(axis=0)

    # Per-edge split: knots with t <= xmin(i) never truncate -> fold into
    # the poly channels; t >= xmax(i) contribute nothing; the rest are
    # relu features, chunked by knot index m.
    ii = np.tile(np.arange(N_IN), N_OUT)
    polyM = knots <= xmin[ii][:, None]                # (EDGES, 15)
    activeM = knots < xmax[ii][:, None]
    reluM = activeM & ~polyM
    relu_cols = np.nonzero(reluM.any(axis=0))[0]

    # Mirror identity relu(x-t)^3 = (x-t)^3 + relu(t-x)^3: flip each relu
    # feature to whichever side has the smaller amplitude, so every chunk
    # is fp16-safe (measured rel err ~3.8e-3 vs the 2e-2 budget).  The
    # (x-t)^3 of each mirrored knot folds into the poly channels.
    amp_unm = np.maximum(xmax[:, None] - t_col[:, relu_cols], 0.0) ** 3
    amp_mir = np.maximum(t_col[:, relu_cols] - xmin[:, None], 0.0) ** 3
    mirror = amp_mir.max(axis=0) < amp_unm.max(axis=0)
    cols_l = relu_cols[mirror]                        # mirrored chunks
    cols_r = relu_cols[~mirror]                       # plain chunks
    nfl, nfr = len(cols_l), len(cols_r)

    # Poly coefficients: original poly knots plus the expanded cubic of
    # every mirrored knot (float64).
    a4 = np.zeros((EDGES, 4))

    def fold_cubic(b, t):
        a4[:, 0] += b * (-t ** 3)
        a4[:, 1] += b * (3 * t ** 2)
        a4[:, 2] += b * (-3 * t)
        a4[:, 3] += b

    for m in range(N_KNOTS):
        fold_cubic(beta[:, m] * polyM[:, m], knots[:, m].astype(np.float64))
    for m in cols_l:
        fold_cubic(beta[:, m] * reluM[:, m], knots[:, m].astype(np.float64))
    cs = c_spl.reshape(EDGES).astype(np.float64)
    Wpoly = cs[:, None] * a4                          # (EDGES, 4)
    bias_fold = bias.astype(np.float64) + \
        Wpoly[:, 0].reshape(N_OUT, N_IN).sum(axis=1)  # a0 folded into bias
    WrFull = (cs[:, None] * beta) * reluM             # (EDGES, 15)

    def chunkT(w):                                    # (EDGES,) -> (N_IN, N_OUT)
        return np.ascontiguousarray(
            w.reshape(N_OUT, N_IN).T.astype(np.float32))

    w32_host = np.ascontiguousarray(np.concatenate(
        [chunkT(Wpoly[:, 1]), chunkT(Wpoly[:, 2]), chunkT(Wpoly[:, 3])],
        axis=1), dtype=np.float32)
    w16a_chunks = [np.ascontiguousarray(c_res.T.astype(np.float16))]
    for m in cols_l:
        w16a_chunks.append(chunkT(WrFull[:, m]).astype(np.float16))
    w16a_host = np.ascontiguousarray(
        np.concatenate(w16a_chunks, axis=1), dtype=np.float16)
    if nfr:
        w16b_host = np.ascontiguousarray(np.concatenate(
            [chunkT(WrFull[:, m]).astype(np.float16) for m in cols_r],
            axis=1), dtype=np.float16)
    else:
        w16b_host = np.zeros((N_IN, N_OUT), np.float16)

    # xp pack: [x.T shard | +t(mirrored) | -t(plain) | bias | zero pad]
    ntl = np.ascontiguousarray(t_col[:, cols_l], dtype=np.float32)
    ntr = np.ascontiguousarray(-t_col[:, cols_r], dtype=np.float32)
    xT = np.ascontiguousarray(x.T, dtype=np.float32)  # (N_IN, BATCH)
    npad = 128 - BSHARD - nfl - nfr
    pad = np.zeros((N_IN, npad), np.float32)
    pad[:, 0] = bias_fold.astype(np.float32)  # per-partition (= per-o) bias

    key = (nfl, nfr)
    if key not in _COMPILED:
        _COMPILED[key] = _build_program(nfl, nfr)
    nc = _COMPILED[key]

    from concourse.bass_utils import run_bass_kernel_spmd
    core_ids = list(range(N_CORES))
    in_maps = []
    for c in core_ids:
        pack = np.concatenate(
            [xT[:, c * BSHARD:(c + 1) * BSHARD], ntl, ntr, pad], axis=1)
        in_maps.append({"xp": np.ascontiguousarray(pack),
                        "w32": w32_host, "w16a": w16a_host,
                        "w16b": w16b_host})
    res = run_bass_kernel_spmd(nc, in_maps, core_ids)
    y_oT = np.concatenate([res.results[c]["y"] for c in core_ids], axis=1)
    return np.ascontiguousarray(y_oT.T.astype(np.float32))
